# revision 35
# baseline (speedup 1.0000x reference)
"""Trainium2 Bass kernel for nn_Bert_Traj_Model (12-layer BERT-style encoder).

Sharding: pure data-parallel over batch. 32 sequences -> 4 per core x 8 cores.
Each core runs the full 12-layer transformer on its 4 sequences (2048 tokens).
No collectives; the host splits inputs and concatenates outputs.

Device layout: residual stream kept TRANSPOSED, h[d_model(6x128 part tiles),
token] in f32r, so every matmul's contraction dim (d) sits on partitions.
LayerNorm outputs are materialized once per 512-token chunk as bf16 tiles
(hn for LN1 -> QKV, hn2 for LN2 -> FFN): per-token -mean / rstd rows are
computed via ones-vector f32r matmuls (partition reduction on the PE),
broadcast across partitions with K=1 outer-product matmuls, and applied with
vector ops. All dense matmuls then run in bf16 with no correction terms.

Pipelining: LN1 stats for layer l+1 chunk c are computed inside layer l's
FFN (chunk c+1's W1 pass); hn for chunk c+1 materializes during chunk c's
attention; all PSUM lives in one hoisted 8-bank pool with phase-disjoint tag
reuse so no cross-phase bank-alias stalls occur. The FFN accumulates its
output in two 3-bank halves (relu activations are retained in SBUF),
freeing banks for the pipelined stats.

Attention: scores computed transposed S^T[k, q] per (seq, head); causal/
prefix mask added in PSUM via identity-matmul on diagonal 128x128 tiles only
(other tiles are fully live or fully masked -> skipped); padding mask is a
per-partition bias on the Exp activation. Softmax runs without max
subtraction (pre-norm LN bounds the scores). The denominator comes free from
an appended ones-column on V ([V|1] stationary); normalization is a
reciprocal row + outer-product broadcast + one multiply.
"""

import numpy as np
from contextlib import ExitStack

import ml_dtypes

import concourse.bass as bass  # noqa: F401  (kept for users of this module)
import concourse.bacc as bacc
import concourse.tile as tile
import concourse.mybir as mybir
from concourse import bass_utils

# ---------------- model constants (hardcoded per spec) ----------------
D = 768
H = 12
DH = 64
FF = 3072
S = 512
B_GLOBAL = 32
N_CORES = 8
BL = B_GLOBAL // N_CORES      # 4 sequences per core
T = BL * S                    # 2048 tokens per core
KT = D // 128                 # 6 d_model partition tiles
FT = FF // 128                # 24 d_ff partition tiles
G16 = T // 128                # 16 token slices per core
NL = 12
EPS = 1e-6
NEGM = -1.0e5                 # additive mask value
TOKV = 10000
TIMV = 48

F32 = mybir.dt.float32
F32R = mybir.dt.float32r
BF16 = mybir.dt.bfloat16
I16 = mybir.dt.int16
AF = mybir.ActivationFunctionType
OP = mybir.AluOpType


def _r(ap):
    return ap.bitcast(F32R)


def build_nc(n_layers=NL, reps=1):
    """Build and compile the per-core Bass program.

    reps > 1 emits the whole forward pass that many times back-to-back —
    used by the benchmark to measure steady-state per-execution time as a
    slope (eliminates the ~80ms axon dispatch round-trip from the
    measurement)."""
    nc = bacc.Bacc("TRN2", target_bir_lowering=False, debug=False)

    d = {}
    def din(name, shape, dt):
        d[name] = nc.dram_tensor(name, shape, dt, kind="ExternalInput").ap()

    din("tok_emb", [TOKV, D], F32)
    din("tim_emb", [TIMV, D], F32)
    din("tok_idx", [128, T // 16], I16)
    din("tim_idx", [128, T // 16], I16)
    din("posT", [128, KT, S], F32)
    din("padadd", [128, G16], F32)
    din("mask00", [128, 128], BF16)
    din("maskdg", [128, 128], BF16)
    din("i128b", [128, 128], BF16)
    din("i128f", [128, 128], F32)
    # weights, all bf16
    din("wqk", [n_layers, 128, 2, KT, KT, 128], BF16)   # [l,dsub,(q|k),n,kt,128]
    din("wv", [n_layers, 128, KT, D], BF16)
    din("wo", [n_layers, 128, KT, D], BF16)
    din("w1t", [n_layers, FT, 128, KT, 128], BF16)
    din("w2t", [n_layers, FT, 128, D], BF16)
    d_out = nc.dram_tensor("out", [T, D], F32, kind="ExternalOutput").ap()

    with tile.TileContext(nc) as tc:
        for _ in range(reps):
            _emit(tc, n_layers, d, d_out)

    nc.compile()
    return nc


def _ln_rows(nc, ps_pool, pstag, sqpool, rowpool, ones_col, eps_row, h, cs,
             nm_tag, rs_tag, scrpool, rbufs=BL):
    """Partition-axis LN stats for h[:, :, cs] -> (-mean, rstd) rows [1, S].
    Stats stream h as f32r (full PE rate at free>=256); two separate psum
    tiles because f32r matmuls must write psum base partition 0."""
    st_s = ps_pool.tile([2, S], F32, tag=pstag, name="st_s")
    st_q = ps_pool.tile([2, S], F32, tag=pstag, name="st_q")
    for kt in range(KT):
        sq = sqpool.tile([128, S], F32R, tag="sq", name="sq")
        nc.scalar.activation(sq[:], h[:, kt, cs], AF.Square)
        nc.tensor.matmul(st_s[:], ones_col[:], h[:, kt, cs],
                         start=(kt == 0), stop=(kt == KT - 1))
        nc.tensor.matmul(st_q[:], ones_col[:], _r(sq[:]),
                         start=(kt == 0), stop=(kt == KT - 1))
    nm = rowpool.tile([1, S], F32R, tag=nm_tag, name="nm", bufs=rbufs)
    nc.vector.tensor_scalar(nm[:], st_s[0:1, :], -1.0 / D, None, OP.mult)
    vs = scrpool.tile([1, S], F32, tag="scr", name="vs")
    nc.vector.tensor_scalar(vs[:], st_q[0:1, :], 1.0 / D, None, OP.mult)
    m2 = scrpool.tile([1, S], F32, tag="scr", name="m2")
    nc.vector.tensor_tensor(m2[:], nm[:], nm[:], OP.mult)
    var = scrpool.tile([1, S], F32, tag="scr", name="var")
    nc.vector.tensor_tensor(var[:], vs[:], m2[:], OP.subtract)
    lnv = scrpool.tile([1, S], F32, tag="scr", name="lnv")
    nc.scalar.activation(lnv[:], var[:], AF.Ln, bias=eps_row[0:1, 0:1])
    rstd = rowpool.tile([1, S], F32R, tag=rs_tag, name="rstd", bufs=rbufs)
    nc.scalar.activation(rstd[:], lnv[:], AF.Exp, scale=-0.5)
    return nm, rstd


def _materialize(nc, ps_pool, pstag, ones_row, h, cs, nm, rstd, hn_dst,
                 bpool, btag):
    """hn_dst[:, kt, :] = (h[:, kt, cs] + bcast(nm)) * bcast(rstd) in bf16."""
    bcn = ps_pool.tile([128, S], F32, tag=pstag, name="bcn")
    nc.tensor.matmul(bcn[:], ones_row[:], nm[:])
    bcr = ps_pool.tile([128, S], F32, tag=pstag, name="bcr")
    nc.tensor.matmul(bcr[:], ones_row[:], rstd[:])
    bcn_s = bpool.tile([128, S], F32, tag=btag, name="bcn_s", bufs=2)
    nc.vector.tensor_copy(bcn_s[:], bcn[:])
    bcr_s = bpool.tile([128, S], F32, tag=btag, name="bcr_s", bufs=2)
    nc.vector.tensor_copy(bcr_s[:], bcr[:])
    for kt in range(KT):
        t1 = bpool.tile([128, S], F32, tag="mat_t1", name="t1", bufs=1)
        nc.gpsimd.tensor_tensor(t1[:], h[:, kt, cs], bcn_s[:], OP.add)
        nc.gpsimd.tensor_tensor(hn_dst[:, kt, :], t1[:], bcr_s[:], OP.mult)


def _emit(tc, n_layers, d, d_out):
    nc = tc.nc
    with ExitStack() as ctx:
        # ---------------- persistent pools ----------------
        cpool = ctx.enter_context(tc.tile_pool(name="const", bufs=1))
        rowp = ctx.enter_context(tc.tile_pool(name="rows", bufs=2))
        scrp = ctx.enter_context(tc.tile_pool(name="scr", bufs=3))
        sqpool = ctx.enter_context(tc.tile_pool(name="sq", bufs=1))
        hpool = ctx.enter_context(tc.tile_pool(name="hres", bufs=1))
        hnpool = ctx.enter_context(tc.tile_pool(name="hn", bufs=2))
        bpool = ctx.enter_context(tc.tile_pool(name="bcast", bufs=2))
        # single hoisted PSUM pool: 8 banks, tags reused across phases
        psp = ctx.enter_context(
            tc.tile_pool(name="ps", bufs=2, space="PSUM"))

        onesf = cpool.tile([128, 128], F32, tag="onesf", name="onesf")
        nc.vector.memset(onesf[:], 1.0)
        ones_col = cpool.tile([128, 2], F32R, tag="onec", name="ones_col")
        nc.vector.tensor_copy(ones_col[:], onesf[:, 0:2])
        ones_row = cpool.tile([1, 128], F32R, tag="oner", name="ones_row")
        nc.vector.tensor_copy(ones_row[:], onesf[0:1, :])
        eps_row = cpool.tile([1, 1], F32, tag="eps", name="eps_row")
        nc.vector.memset(eps_row[:], EPS)
        i128b = cpool.tile([128, 128], BF16, tag="i128b", name="i128b")
        nc.sync.dma_start(i128b[:], d["i128b"])
        i128f = cpool.tile([128, 128], F32, tag="i128f", name="i128f")
        nc.sync.dma_start(i128f[:], d["i128f"])
        mask00 = cpool.tile([128, 128], BF16, tag="m00", name="mask00")
        nc.sync.dma_start(mask00[:], d["mask00"])
        maskdg = cpool.tile([128, 128], BF16, tag="mdg", name="maskdg")
        nc.sync.dma_start(maskdg[:], d["maskdg"])
        padsb = cpool.tile([128, G16], F32, tag="pad", name="padsb")
        nc.sync.dma_start(padsb[:], d["padadd"])

        h = hpool.tile([128, KT, T], F32R, tag="h", name="h")
        vaug = cpool.tile([128, BL, H, DH + 1], BF16, tag="vaug", name="vaug")
        nc.vector.tensor_copy(
            vaug[:, :, :, DH:DH + 1],
            onesf[:, 0:BL * H].rearrange("p (b h) -> p b h ()", b=BL))

        # ================= embedding (two halves to bound SBUF) ============
        G8 = G16 // 2
        with tc.tile_pool(name="emb", bufs=1) as ep:
            post = ep.tile([128, KT, S], F32, tag="post", name="post")
            nc.sync.dma_start(post[:], d["posT"])
            tokidx = ep.tile([128, T // 16], I16, tag="tokidx", name="tokidx")
            timidx = ep.tile([128, T // 16], I16, tag="timidx", name="timidx")
            nc.sync.dma_start(tokidx[:], d["tok_idx"])
            nc.sync.dma_start(timidx[:], d["tim_idx"])
            for half in range(2):
                tokn = ep.tile([128, G8, D], F32, tag="tokn", name="tokn")
                timn = ep.tile([128, G8, D], F32, tag="timn", name="timn")
                isl = slice(half * (T // 32), (half + 1) * (T // 32))
                nc.gpsimd.dma_gather(tokn[:], d["tok_emb"], tokidx[:, isl],
                                     num_idxs=T // 2, num_idxs_reg=T // 2,
                                     elem_size=D)
                nc.gpsimd.dma_gather(timn[:], d["tim_emb"], timidx[:, isl],
                                     num_idxs=T // 2, num_idxs_reg=T // 2,
                                     elem_size=D)
                for gg in range(G8):
                    g = half * G8 + gg
                    sl = (g % BL) * 128
                    for kt in range(KT):
                        p = psp.tile([128, 128], F32,
                                     tag=("mm", "sp", "ops")[kt % 3],
                                     name="etp")
                        nc.tensor.matmul(p[:],
                                         tokn[:, gg, kt * 128:(kt + 1) * 128],
                                         i128f[:], is_transpose=True,
                                         start=True, stop=False)
                        nc.tensor.matmul(p[:],
                                         timn[:, gg, kt * 128:(kt + 1) * 128],
                                         i128f[:], is_transpose=True,
                                         start=False, stop=True)
                        nc.vector.tensor_tensor(
                            h[:, kt, g * 128:(g + 1) * 128], p[:],
                            post[:, kt, sl:sl + 128], OP.add)

        # embedding layernorm (writes h in place), then LN1 stats + hn(0,0)
        for c in range(BL):
            cs = slice(c * S, (c + 1) * S)
            nm, rstd = _ln_rows(nc, psp, "bco", sqpool, rowp, ones_col,
                                eps_row, h, cs, "nm1", "rs1", scrp)
            bcn = psp.tile([128, S], F32, tag="sp", name="ebcn")
            nc.tensor.matmul(bcn[:], ones_row[:], nm[:])
            bcr = psp.tile([128, S], F32, tag="sp", name="ebcr")
            nc.tensor.matmul(bcr[:], ones_row[:], rstd[:])
            bcn_s = bpool.tile([128, S], F32, tag="lnb", name="ebcn_s",
                               bufs=2)
            nc.vector.tensor_copy(bcn_s[:], bcn[:])
            bcr_s = bpool.tile([128, S], F32, tag="lnb", name="ebcr_s",
                               bufs=2)
            nc.vector.tensor_copy(bcr_s[:], bcr[:])
            for kt in range(KT):
                t1 = bpool.tile([128, S], F32, tag="mat_t1", name="et1",
                                bufs=1)
                nc.vector.tensor_tensor(t1[:], h[:, kt, cs], bcn_s[:], OP.add)
                nc.vector.tensor_tensor(h[:, kt, cs], t1[:], bcr_s[:],
                                        OP.mult)

        # LN1 stats for layer 0, all chunks; then hn(0, chunk0)
        nm1 = [None] * BL
        rs1 = [None] * BL
        for c in range(BL):
            cs = slice(c * S, (c + 1) * S)
            nm1[c], rs1[c] = _ln_rows(nc, psp, "bco", sqpool, rowp, ones_col,
                                      eps_row, h, cs, "nm1", "rs1", scrp)
        hn = [None, None]   # ring of 2: hn[c % 2]
        hn[0] = hnpool.tile([128, KT, S], BF16, tag="hn", name="hn0", bufs=2)
        _materialize(nc, psp, "sp", ones_row, h, slice(0, S),
                     nm1[0], rs1[0], hn[0], bpool, "lnb")

        # ================= transformer layers =================
        lctx = ctx.enter_context(ExitStack())
        wqkp = lctx.enter_context(tc.tile_pool(name="wqk", bufs=3))
        wvop = lctx.enter_context(tc.tile_pool(name="wvo", bufs=1))
        hn2pool = lctx.enter_context(tc.tile_pool(name="hn2", bufs=BL))

        def wqk_dma(lyr_, n_):
            t = wqkp.tile([128, 2, KT, 128], BF16, tag="wqk", name="wqkn")
            nc.sync.dma_start(t[:], d["wqk"][lyr_, :, :, n_])
            return t

        wv = wvop.tile([128, KT, D], BF16, tag="wv", name="wv")
        nc.sync.dma_start(wv[:], d["wv"][0])
        wo = wvop.tile([128, KT, D], BF16, tag="wo", name="wo")
        nc.sync.dma_start(wo[:], d["wo"][0])

        w1p = lctx.enter_context(tc.tile_pool(name="w1", bufs=4))
        w2ap = lctx.enter_context(tc.tile_pool(name="w2a", bufs=4))
        w2bp = lctx.enter_context(tc.tile_pool(name="w2b", bufs=4))
        rlp = lctx.enter_context(tc.tile_pool(name="rl", bufs=1))
        qkp = lctx.enter_context(tc.tile_pool(name="qk", bufs=2))
        atp = lctx.enter_context(tc.tile_pool(name="at", bufs=4))
        oTp = lctx.enter_context(tc.tile_pool(name="oT", bufs=1))
        bcop = lctx.enter_context(tc.tile_pool(name="bcos", bufs=1))

        for lyr in range(n_layers):
            # ---------- phase A: QKV + attention + Wo + LN2 ----------
            nm2 = [None] * BL
            rs2 = [None] * BL
            hn2 = [None] * BL
            for c in range(BL):
                cs = slice(c * S, (c + 1) * S)

                # V in natural layout into [V|1] aug slots
                for g in range(BL):
                    tok = slice(c * S + g * 128, c * S + (g + 1) * 128)
                    for half in range(2):
                        n0 = half * 384
                        ps = psp.tile([128, 384], F32, tag="mm", name="vps")
                        for kt in range(KT):
                            nc.tensor.matmul(
                                ps[:], hn[c % 2][:, kt, g * 128:(g + 1) * 128],
                                wv[:, kt, n0:n0 + 384],
                                start=(kt == 0), stop=(kt == KT - 1))
                        h0 = half * 6
                        nc.vector.tensor_copy(
                            vaug[:, g, h0:h0 + 6, 0:DH],
                            ps[:].rearrange("p (h d) -> p h d", d=DH))

                # materialize hn for chunk c+1 (overlaps attention below)
                if c + 1 < BL:
                    hn[(c + 1) % 2] = hnpool.tile([128, KT, S], BF16,
                                                  tag="hn", name="hnN",
                                                  bufs=2)
                    _materialize(nc, psp, "sp", ones_row, h,
                                 slice((c + 1) * S, (c + 2) * S),
                                 nm1[c + 1], rs1[c + 1], hn[(c + 1) % 2],
                                 bpool, "lnb")

                # per n-tile: Q, K, then attention for heads 2n, 2n+1
                oT = oTp.tile([128, KT, S], BF16, tag="oT", name="oT")
                wqk_t = [wqk_dma(lyr, 0), wqk_dma(lyr, 1)] + [None] * (KT - 2)
                for n in range(KT):
                    if n + 2 < KT:
                        wqk_t[n + 2] = wqk_dma(lyr, n + 2)
                    qs = qkp.tile([128, S], BF16, tag="qs", name="qs")
                    ks = qkp.tile([128, S], BF16, tag="ks", name="ks")
                    for dst, qk in ((qs, 0), (ks, 1)):
                        ps = psp.tile([128, S], F32, tag="mm", name="qkps")
                        for kt in range(KT):
                            nc.tensor.matmul(
                                ps[:], wqk_t[n][:, qk, kt, :],
                                hn[c % 2][:, kt, :],
                                start=(kt == 0), stop=(kt == KT - 1))
                        nc.vector.tensor_copy(dst[:], ps[:])

                    for sub in range(2):
                        hd = 2 * n + sub
                        pb = 64 * sub
                        o_ps = psp.tile([DH + 1, S], F32, tag="ops",
                                        name="o_ps")
                        at_tiles = []
                        for j in range(BL):
                            q0 = j * 128
                            sp = psp.tile([128, S], F32, tag="sp", name="sp")
                            nc.tensor.matmul(
                                sp[:, q0:],
                                ks[pb:pb + DH, j * 128:(j + 1) * 128],
                                qs[pb:pb + DH, q0:],
                                start=True, stop=False)
                            nc.tensor.matmul(
                                sp[:, q0:q0 + 128], i128b[:],
                                (mask00 if j == 0 else maskdg)[:],
                                start=False, stop=True)
                            at = atp.tile([128, S], BF16, tag="at", name="at")
                            nc.scalar.activation(
                                at[:, q0:], sp[:, q0:], AF.Exp,
                                bias=padsb[:, c * BL + j:c * BL + j + 1])
                            at_tiles.append(at)
                        for j in range(BL):
                            q0 = j * 128
                            nc.tensor.matmul(
                                o_ps[:, q0:], vaug[:, j, hd, :],
                                at_tiles[j][:, q0:],
                                start=(j == 0), stop=(j == BL - 1))
                        inv = scrp.tile([1, S], F32R, tag="scr", name="inv")
                        with nc.allow_low_precision(
                                reason="f32r softmax denom (19-bit ok)"):
                            nc.vector.reciprocal(inv[:], o_ps[DH:DH + 1, :])
                        bco = psp.tile([DH, S], F32, tag="bco", name="bco")
                        nc.tensor.matmul(bco[:], ones_row[0:1, 0:DH], inv[:])
                        bcos = bcop.tile([DH, S], F32, tag="bcos",
                                         name="bcos")
                        nc.scalar.activation(bcos[:], bco[:], AF.Copy)
                        nc.vector.tensor_tensor(
                            oT[pb:pb + DH, n, :], o_ps[0:DH, :], bcos[:],
                            OP.mult)

                # Wo + residual
                for n in range(KT):
                    ps = psp.tile([128, S], F32, tag="mm", name="wops")
                    for kt in range(KT):
                        nc.tensor.matmul(
                            ps[:], wo[:, kt, n * 128:(n + 1) * 128],
                            oT[:, kt, :],
                            start=(kt == 0), stop=(kt == KT - 1))
                    nc.vector.tensor_tensor(h[:, n, cs], h[:, n, cs],
                                            ps[:], OP.add)

                # LN2 stats for this chunk (hn2 materializes lazily in B)
                nm2[c], rs2[c] = _ln_rows(nc, psp, "sp", sqpool, rowp,
                                          ones_col, eps_row, h, cs,
                                          "nm2", "rs2", scrp)
                if c == 0:
                    # hn2 for chunk 0 early: its DVE chain overlaps the
                    # attention of chunks 1-3 so phase B starts stall-free
                    hn2[0] = hn2pool.tile([128, KT, S], BF16, tag="hn2",
                                          name="hn2_0", bufs=2)
                    _materialize(nc, psp, "sp", ones_row, h, slice(0, S),
                                 nm2[0], rs2[0], hn2[0], bpool, "lnb")

            # ---------- phase B: FFN (+ pipelined next-layer LN1) ----------
            last = lyr == n_layers - 1
            if not last:
                wv = wvop.tile([128, KT, D], BF16, tag="wv", name="wvN")
                nc.sync.dma_start(wv[:], d["wv"][lyr + 1])
                wo = wvop.tile([128, KT, D], BF16, tag="wo", name="woN")
                nc.sync.dma_start(wo[:], d["wo"][lyr + 1])

            for c in range(BL):
                cs = slice(c * S, (c + 1) * S)
                rl = rlp.tile([128, FT, S], BF16, tag="rl", name="rl")

                # lazily materialize hn2 for the NEXT chunk (overlaps FFN)
                if c + 1 < BL:
                    hn2[c + 1] = hn2pool.tile([128, KT, S], BF16, tag="hn2",
                                              name="hn2_n", bufs=2)
                    _materialize(nc, psp, "mm", ones_row, h,
                                 slice((c + 1) * S, (c + 2) * S),
                                 nm2[c + 1], rs2[c + 1], hn2[c + 1],
                                 bpool, "lnb")

                # pass 1: W1 + relu; W2 first n-half accumulation
                w1t_ = [None] * FT
                w2a_ = [None] * FT
                w2b_ = [None] * FT
                for f in range(2):
                    w1t_[f] = w1p.tile([128, KT, 128], BF16, tag="w1",
                                       name="w1", bufs=3)
                    nc.sync.dma_start(w1t_[f][:], d["w1t"][lyr, f])
                    w2a_[f] = w2ap.tile([128, 384], BF16, tag="w2a",
                                        name="w2a", bufs=3)
                    nc.sync.dma_start(w2a_[f][:], d["w2t"][lyr, f, :, 0:384])
                fh = [psp.tile([128, S], F32, tag=("sp", "sp", "ops")[n],
                               name="fh") for n in range(3)]
                for f in range(FT):
                    if f + 2 < FT:
                        w1t_[f + 2] = w1p.tile([128, KT, 128], BF16, tag="w1",
                                               name="w1", bufs=3)
                        nc.sync.dma_start(w1t_[f + 2][:],
                                          d["w1t"][lyr, f + 2])
                        w2a_[f + 2] = w2ap.tile([128, 384], BF16, tag="w2a",
                                                name="w2a", bufs=3)
                        nc.sync.dma_start(w2a_[f + 2][:],
                                          d["w2t"][lyr, f + 2, :, 0:384])
                    ps1 = psp.tile([128, S], F32, tag="mm", name="ps1")
                    for kt in range(KT):
                        nc.tensor.matmul(ps1[:], w1t_[f][:, kt, :],
                                         hn2[c][:, kt, :],
                                         start=(kt == 0), stop=(kt == KT - 1))
                    nc.vector.tensor_scalar(rl[:, f, :], ps1[:], 0.0, None, OP.max)
                    for n in range(3):
                        nc.tensor.matmul(
                            fh[n][:], w2a_[f][:, n * 128:(n + 1) * 128],
                            rl[:, f, :], start=(f == 0), stop=(f == FT - 1))
                    # pipelined LN1 stats for layer lyr+1, chunk c-1
                    if f == 2 and not last and c >= 1:
                        nm1[c - 1], rs1[c - 1] = _ln_rows(
                            nc, psp, "bco", sqpool, rowp, ones_col, eps_row,
                            h, slice((c - 1) * S, c * S), "nm1", "rs1", scrp)
                    if f >= FT - 2:
                        fb = f - (FT - 2)
                        w2b_[fb] = w2bp.tile([128, 384], BF16, tag="w2b",
                                             name="w2b", bufs=3)
                        nc.sync.dma_start(w2b_[fb][:],
                                          d["w2t"][lyr, fb, :, 384:768])
                for n in range(3):
                    nc.vector.tensor_tensor(h[:, n, cs], h[:, n, cs],
                                            fh[n][:], OP.add)

                # pass 2: W2 second n-half from retained relu activations
                fh2 = [psp.tile([128, S], F32, tag=("sp", "sp", "ops")[n],
                                name="fh2") for n in range(3)]
                for f in range(FT):
                    if f + 2 < FT:
                        w2b_[f + 2] = w2bp.tile([128, 384], BF16, tag="w2b",
                                                name="w2b", bufs=3)
                        nc.sync.dma_start(w2b_[f + 2][:],
                                          d["w2t"][lyr, f + 2, :, 384:768])
                    for n in range(3):
                        nc.tensor.matmul(
                            fh2[n][:], w2b_[f][:, n * 128:(n + 1) * 128],
                            rl[:, f, :], start=(f == 0), stop=(f == FT - 1))
                for n in range(3):
                    nc.vector.tensor_tensor(h[:, 3 + n, cs], h[:, 3 + n, cs],
                                            fh2[n][:], OP.add)

            if not last:
                # trailing LN1 stats for chunk 3, then hn(lyr+1, 0)
                nm1[BL - 1], rs1[BL - 1] = _ln_rows(
                    nc, psp, "bco", sqpool, rowp, ones_col, eps_row,
                    h, slice((BL - 1) * S, BL * S), "nm1", "rs1", scrp)
                hn[0] = hnpool.tile([128, KT, S], BF16, tag="hn", name="hn0N",
                                    bufs=2)
                _materialize(nc, psp, "mm", ones_row, h, slice(0, S),
                             nm1[0], rs1[0], hn[0], bpool, "lnb")

        # ================= output transpose =================
        lctx.close()
        with tc.tile_pool(name="outsb", bufs=2) as osb:
            for g in range(G16):
                ob = osb.tile([128, D], F32, tag="ob", name="ob")
                for kt in range(KT):
                    p = psp.tile([128, 128], F32,
                                 tag=("mm", "sp", "ops")[kt % 3], name="otp")
                    nc.tensor.matmul(p[:],
                                     h[:, kt,
                                       g * 128:(g + 1) * 128].bitcast(F32),
                                     i128f[:], is_transpose=True)
                    nc.vector.tensor_copy(ob[:, kt * 128:(kt + 1) * 128],
                                          p[:])
                nc.sync.dma_start(d_out[g * 128:(g + 1) * 128, :], ob[:])


# ======================= host side =======================

def _pos_enc():
    pos = np.arange(S, dtype=np.float32)[:, None]
    i = np.arange(0, D, 2, dtype=np.float32)[None, :]
    ang = pos / np.power(10000.0, i / D)
    pe = np.zeros((S, D), dtype=np.float32)
    pe[:, 0::2] = np.sin(ang)
    pe[:, 1::2] = np.cos(ang)
    return pe


def _idx16(v):
    """dma_gather index layout: idx i at [i % 16, i // 16].
    CoreSim's ucode model reads partitions 0..15; the deployed HW ucode reads
    partitions 16..31 — write both ranges so either consumer sees the same
    indices."""
    arr = np.zeros((128, T // 16), np.int16)
    w = v.reshape(T // 16, 16).T.astype(np.int16)
    arr[:16, :] = w
    arr[16:32, :] = w
    return arr


_NC_CACHE = {}


def _get_nc(nl, reps=1):
    key = (nl, reps)
    if key not in _NC_CACHE:
        _NC_CACHE[key] = build_nc(nl, reps=reps)
    return _NC_CACHE[key]


def prepare(inputs, n_layers=None):
    """Host-side preprocessing -> (nl, per-core input maps)."""
    x = np.asarray(inputs["x"])
    time_t = np.asarray(inputs["time"])
    len_traj = int(np.asarray(inputs["len_traj"]))
    tok_emb = np.asarray(inputs["tok_emb"], np.float32)
    time_emb = np.asarray(inputs["time_emb"], np.float32)
    emb_g = np.asarray(inputs["emb_g"], np.float32)
    emb_b = np.asarray(inputs["emb_b"], np.float32)
    Wq = np.asarray(inputs["Wq"], np.float32)
    bq = np.asarray(inputs["bq"], np.float32)
    Wk = np.asarray(inputs["Wk"], np.float32)
    bk = np.asarray(inputs["bk"], np.float32)
    Wv = np.asarray(inputs["Wv"], np.float32)
    bv = np.asarray(inputs["bv"], np.float32)
    Wo = np.asarray(inputs["Wo"], np.float32)
    bo = np.asarray(inputs["bo"], np.float32)
    ln1_g = np.asarray(inputs["ln1_g"], np.float32)
    ln1_b = np.asarray(inputs["ln1_b"], np.float32)
    W1 = np.asarray(inputs["W1"], np.float32)
    b1 = np.asarray(inputs["b1"], np.float32)
    W2 = np.asarray(inputs["W2"], np.float32)
    b2 = np.asarray(inputs["b2"], np.float32)
    ln2_g = np.asarray(inputs["ln2_g"], np.float32)
    ln2_b = np.asarray(inputs["ln2_b"], np.float32)

    nl = Wq.shape[0] if n_layers is None else n_layers
    Wq, Wk, Wv, Wo = Wq[:nl], Wk[:nl], Wv[:nl], Wo[:nl]
    bq, bk, bv, bo = bq[:nl], bk[:nl], bv[:nl], bo[:nl]
    W1, b1, W2, b2 = W1[:nl], b1[:nl], W2[:nl], b2[:nl]
    ln1_g, ln1_b, ln2_g, ln2_b = ln1_g[:nl], ln1_b[:nl], ln2_g[:nl], ln2_b[:nl]

    scale = np.float32(1.0 / np.sqrt(DH))

    # fold LN gains into adjacent weights (exact); betas must be zero
    Wq_f = ln1_g[:, :, None] * Wq * scale
    Wk_f = ln1_g[:, :, None] * Wk
    Wv_f = ln1_g[:, :, None] * Wv
    W1_f = ln2_g[:, :, None] * W1

    def _mx(a):
        return np.abs(a).max() if a.size else 0.0

    unsupported = []
    if _mx(ln1_b) > 0 or _mx(ln2_b) > 0:
        unsupported.append("ln betas")
    if _mx(bq) > 0 or _mx(bk) > 0 or _mx(bv) > 0 or _mx(b1) > 0:
        unsupported.append("bq/bk/bv/b1")
    if _mx(bo) > 0 or _mx(b2) > 0:
        unsupported.append("bo/b2")
    if np.abs(emb_g - 1).max() > 0 or _mx(emb_b) > 0:
        unsupported.append("emb g/b")
    if unsupported:
        raise NotImplementedError(f"nonzero params unsupported: {unsupported}")

    bf = ml_dtypes.bfloat16

    def qlay(w):  # [L, D, N] -> [L, 128, KT, N]
        return np.ascontiguousarray(
            w.reshape(nl, KT, 128, w.shape[2]).transpose(0, 2, 1, 3)
        ).astype(bf)

    def nlay(w):  # [L, D, N] -> [L, N/128, 128(d sub), D/128, 128(n sub)]
        nt = w.shape[2] // 128
        return np.ascontiguousarray(
            w.reshape(nl, KT, 128, nt, 128).transpose(0, 3, 2, 1, 4)
        ).astype(bf)

    # wqk: [l, 128(dsub), (q|k), n, kt, 128]
    wq_n = np.asarray(
        Wq_f.reshape(nl, KT, 128, KT, 128).transpose(0, 2, 3, 1, 4))
    wk_n = np.asarray(
        Wk_f.reshape(nl, KT, 128, KT, 128).transpose(0, 2, 3, 1, 4))
    wqk = np.ascontiguousarray(
        np.stack([wq_n, wk_n], axis=2)).astype(bf)

    shared = {
        "tok_emb": tok_emb, "tim_emb": time_emb,
        "wqk": wqk, "wv": qlay(Wv_f), "wo": qlay(Wo),
        "w1t": nlay(W1_f),
        "w2t": np.ascontiguousarray(W2.reshape(nl, FT, 128, D)).astype(bf),
    }

    pe = _pos_enc()
    shared["posT"] = np.ascontiguousarray(
        pe.T.reshape(KT, 128, S).transpose(1, 0, 2))

    ii = np.arange(128)
    tril = (ii[None, :] >= ii[:, None])  # [k, q]: q >= k
    shared["maskdg"] = np.where(tril, 0.0, NEGM).astype(bf)
    shared["mask00"] = np.where(tril | (ii[:, None] < len_traj), 0.0,
                                NEGM).astype(bf)
    eye = np.eye(128, dtype=np.float32)
    shared["i128b"] = eye.astype(bf)
    shared["i128f"] = eye

    in_maps = []
    for core in range(N_CORES):
        bs = slice(core * BL, (core + 1) * BL)
        xl = np.asarray(x[bs]).reshape(-1)
        tl = np.asarray(time_t[bs]).reshape(-1)
        pad = np.where(xl > 0, 0.0, NEGM).astype(np.float32)
        m = dict(shared)
        m["tok_idx"] = _idx16(xl)
        m["tim_idx"] = _idx16(tl)
        m["padadd"] = np.ascontiguousarray(pad.reshape(G16, 128).T)
        in_maps.append(m)

    return nl, in_maps


def run(inputs, n_layers=None, reps=1, **run_kwargs):
    """Run on hardware; returns (output [32, 512, 768], BassKernelResults)."""
    nl, in_maps = prepare(inputs, n_layers)
    nc = _get_nc(nl, reps)
    res = bass_utils.run_bass_kernel_spmd(nc, in_maps,
                                          core_ids=list(range(N_CORES)),
                                          **run_kwargs)
    outs = [res.results[i]["out"] for i in range(N_CORES)]
    full = np.concatenate([np.asarray(o).reshape(BL, S, D) for o in outs],
                          axis=0)
    return full.astype(np.float32), res


def kernel(**inputs):
    out, _ = run(inputs)
    return out
